# revision 1
# baseline (speedup 1.0000x reference)
"""Trainium2 Bass kernel for CrossAttention (B=4, T=2048, S=4096, D=256, H=8, Dh=32).

Sharding: 8 cores = 4 batches x 2 T-halves (each core owns 1024 query rows of
one batch, all heads). No collectives needed: each core computes its full
output rows; host concatenates.

Per-core dataflow (all "T"-like dims on the free axis, contractions on
partitions):
  xT [256, 1024], cT [256, 4096] via PE transposes (fp32 in, fp16 out)
  qT = w_q^T @ xT     [256(hid), 1024]
  kT = w_k^T @ cT     [256(hid), 4096]
  v  = cT^T @ w_v     [4096(S), 256] stored interleaved with a ones column
                      per head (v' [S, 33] per head) so attn@v' also yields
                      the softmax denominator for free.
  per (head, S-tile of 128):
    sT = kT_h_tile^T @ qT_h   [128(S), 1024(T)]  (fp16 matmul -> PSUM fp32)
    attnT = exp(sT * scale)   (ScalarE, PSUM->SBUF fp16)
    outT'_h += v'_h_tile^T @ attnT   [33, 1024] accumulated in PSUM fp32
  normalize outT by the broadcast reciprocal denominator,
  out = outT^T @ w_out + b_out.

Matmuls use fp16 operands (1 cycle/row; fp32 is 4x) with fp32 PSUM
accumulation. The structure is shaped by a hardware constraint: a PE
instruction can carry only ONE semaphore wait, so every matmul is arranged
to depend on at most one other engine (one shared PSUM pool, accumulator
dumps on ScalarE so slot releases merge with the exp waits, etc).
"""

import sys

if "/opt/trn_rl_repo" not in sys.path:
    sys.path.insert(0, "/opt/trn_rl_repo")

from contextlib import ExitStack

import numpy as np

import concourse.bass as bass
import concourse.tile as tile
from concourse import bacc
from concourse import mybir
from concourse.bass_utils import run_bass_kernel_spmd

B, T, S, D, H, Dh = 4, 2048, 4096, 256, 8, 32
TL = T // 2  # 1024 query rows per core
NXT = TL // 128  # 8 x tiles
SCALE = Dh ** -0.5
FP = mybir.dt.float32
F16 = mybir.dt.float16
NST = S // 128  # 32 S-tiles
VW = H * (Dh + 1)  # 264 packed v' columns per S-tile
# head h -> (triple tile, 32-row block): heads grouped 3+3+2 so every row
# block starts at partition 0/32/64 (hardware base-partition constraint)
TRIP = [(h // 3, h % 3) for h in range(H)]
TRIP_HEADS = [[0, 1, 2], [3, 4, 5], [6, 7]]


def build_bass():
    nc = bacc.Bacc()
    ident_d = nc.declare_dram_parameter("ident", [128, 128], FP, isOutput=False)
    x_d = nc.declare_dram_parameter("x", [TL, D], FP, isOutput=False)
    ctx_d = nc.declare_dram_parameter("context", [S, D], FP, isOutput=False)
    wq_d = nc.declare_dram_parameter("w_q", [D, D], FP, isOutput=False)
    wkv_d = nc.declare_dram_parameter("w_kv", [D, 2 * D], FP, isOutput=False)
    wout_d = nc.declare_dram_parameter("w_out", [D, D], FP, isOutput=False)
    bout_d = nc.declare_dram_parameter("b_out", [1, D], FP, isOutput=False)
    out_d = nc.declare_dram_parameter("out", [TL, D], FP, isOutput=True)
    dnscr = nc.dram_tensor("dnscratch", [H, TL], FP)

    with tile.TileContext(nc) as tc, ExitStack() as ctx:
        consts = ctx.enter_context(tc.tile_pool(name="consts", bufs=1))
        persist = ctx.enter_context(tc.tile_pool(name="persist", bufs=1))
        psum = ctx.enter_context(tc.tile_pool(name="psum", bufs=3, space="PSUM"))

        identity = consts.tile([128, 128], FP, tag="identity", name="identity")
        # hsel[b] [1, 96]: ones in columns 32b..32b+32 — builds the per-head
        # reciprocal broadcast via K=1 accumulating matmuls
        hsel = []
        for b in range(3):
            m = consts.tile([1, 96], F16, tag=f"hsel{b}", name=f"hsel{b}")
            nc.vector.memset(m, 0.0)
            nc.vector.memset(m[0:1, 32 * b : 32 * b + 32], 1.0)
            hsel.append(m)

        wq = [persist.tile([128, D], F16, tag=f"wq{j}", name=f"wq{j}") for j in range(2)]
        wkv = [persist.tile([128, 2 * D], F16, tag=f"wkv{j}", name=f"wkv{j}") for j in range(2)]
        wo_rows = [96, 96, 64]
        woutg = [
            persist.tile([wo_rows[t], D], F16, tag=f"woutg{t}", name=f"woutg{t}")
            for t in range(3)
        ]
        bias_b = persist.tile([128, D], FP, tag="bias_b", name="bias_b")
        bias_c = persist.tile([128, D], FP, tag="bias_c", name="bias_c")
        # 2 heads per tile (base-partition constraint)
        qT = [persist.tile([64, TL], F16, tag=f"qT{j}", name=f"qT{j}") for j in range(4)]
        kT = [persist.tile([64, S], F16, tag=f"kT{j}", name=f"kT{j}") for j in range(4)]
        vP = persist.tile([128, NST * VW], F16, tag="vP", name="vP")
        dumpT = [
            persist.tile([96, TL], F16, tag=f"dumpT{t}", name=f"dumpT{t}")
            for t in range(3)
        ]
        outTh = [
            persist.tile([96, TL], F16, tag=f"outTh{t}", name=f"outTh{t}")
            for t in range(3)
        ]
        rcp = [persist.tile([96, TL], FP, tag=f"rcp{t}", name=f"rcp{t}") for t in range(3)]
        rden = [persist.tile([96, TL], FP, tag=f"rden{t}", name=f"rden{t}") for t in range(3)]
        xT = [persist.tile([128, TL], F16, tag=f"xT{j}", name=f"xT{j}") for j in range(2)]
        cT = [persist.tile([128, S], F16, tag=f"cT{j}", name=f"cT{j}") for j in range(2)]
        early = tc.tile_pool(name="early", bufs=1)
        ep = early.__enter__()
        x_all = ep.tile([128, NXT, D], FP, tag="x_all", name="x_all")
        c_all = ep.tile([128, NST, D], FP, tag="c_all", name="c_all")
        wstage = [
            ep.tile([128, 3 * D], FP, tag=f"wstage{j}", name=f"wstage{j}")
            for j in range(2)
        ]
        wso = [
            ep.tile([wo_rows[t], D], FP, tag=f"wso{t}", name=f"wso{t}")
            for t in range(3)
        ]

        # ---- Phase 0: loads + fp16 weight conversion ----
        # DMA issue order is load-bearing: the HW DGE queue is assigned
        # round-robin (mod 8) over DMA program order. ident is DMA #0 and
        # x_all #8 (same queue), c_all #9 -> the two first-touch transposes
        # each carry exactly one queue wait, and PE never needs a second one.
        nc.sync.dma_start(out=identity, in_=ident_d[:, :])
        for j in range(2):
            nc.sync.dma_start(out=wstage[j][:, 0:D], in_=wq_d[128 * j : 128 * j + 128, :])
            nc.sync.dma_start(
                out=wstage[j][:, D : 3 * D], in_=wkv_d[128 * j : 128 * j + 128, :]
            )
            nc.vector.tensor_copy(wq[j], wstage[j][:, 0:D])
            nc.vector.tensor_copy(wkv[j], wstage[j][:, D : 3 * D])
        ro = 0
        for t in range(3):
            nc.sync.dma_start(out=wso[t], in_=wout_d[ro : ro + wo_rows[t], :])
            nc.vector.tensor_copy(woutg[t], wso[t])
            ro += wo_rows[t]
        nc.sync.dma_start(out=x_all, in_=x_d.rearrange("(t p) d -> p t d", p=128))
        ctx_r = ctx_d.rearrange("(t p) d -> p t d", p=128)
        for cc in range(4):
            nc.sync.dma_start(out=c_all[:, 8 * cc : 8 * cc + 8, :], in_=ctx_r[:, 8 * cc : 8 * cc + 8, :])
        nc.sync.dma_start(out=bias_b, in_=bout_d[0:1, :].partition_broadcast(128))
        nc.vector.tensor_copy(bias_c, bias_b)

        # ---- Phase 1: transpose x and context ----
        # convert to fp16 first: fp32 PE transposes run 4 cycles/row
        xh = persist.tile([128, NXT, D], F16, tag="xh", name="xh")
        ch = persist.tile([128, NST, D], F16, tag="ch", name="ch")
        idh = consts.tile([128, 128], F16, tag="idh", name="idh")
        nc.vector.tensor_copy(idh, identity)
        nc.vector.tensor_copy(xh, x_all)
        for cc in range(4):
            nc.vector.tensor_copy(ch[:, 8 * cc : 8 * cc + 8, :], c_all[:, 8 * cc : 8 * cc + 8, :])

        tcount = [0]

        def do_transpose(src_all, st, j, dstT):
            pt = psum.tile([128, 128], F16, tag="sc", name="pt")
            nc.tensor.transpose(pt, src_all[:, st, 128 * j : 128 * j + 128], idh)
            if tcount[0] % 2 == 0:
                nc.vector.tensor_copy(dstT[:, 128 * st : 128 * st + 128], pt)
            else:
                nc.scalar.copy(dstT[:, 128 * st : 128 * st + 128], pt)
            tcount[0] += 1

        for t in range(NXT):
            for j in range(2):
                do_transpose(xh, t, j, xT[j])
        for st in range(NST):
            for j in range(2):
                do_transpose(ch, st, j, cT[j])

        early.__exit__(None, None, None)
        attnp = ctx.enter_context(tc.tile_pool(name="attn", bufs=16))
        dnp = ctx.enter_context(tc.tile_pool(name="dnp", bufs=2))
        fstage = ctx.enter_context(tc.tile_pool(name="fstage", bufs=4))

        # ---- Phase 2: projections ----
        def qk_proj(mj):
            for nt in range(TL // 512):
                pq = psum.tile([128, 512], FP, tag="sc", name="pq")
                for kj in range(2):
                    nc.tensor.matmul(
                        pq,
                        lhsT=wq[kj][:, 128 * mj : 128 * mj + 128],
                        rhs=xT[kj][:, 512 * nt : 512 * nt + 512],
                        start=(kj == 0),
                        stop=(kj == 1),
                    )
                for half in range(2):
                    nc.vector.tensor_copy(
                        qT[2 * mj + half][:, 512 * nt : 512 * nt + 512],
                        pq[64 * half : 64 * half + 64, :],
                    )
            for nt in range(S // 512):
                pk = psum.tile([128, 512], FP, tag="sc", name="pk")
                for kj in range(2):
                    nc.tensor.matmul(
                        pk,
                        lhsT=wkv[kj][:, 128 * mj : 128 * mj + 128],
                        rhs=cT[kj][:, 512 * nt : 512 * nt + 512],
                        start=(kj == 0),
                        stop=(kj == 1),
                    )
                nc.vector.tensor_copy(
                    kT[2 * mj][:, 512 * nt : 512 * nt + 512], pk[0:64, :]
                )
                nc.scalar.copy(
                    kT[2 * mj + 1][:, 512 * nt : 512 * nt + 512], pk[64:128, :]
                )

        def v_proj(st_lo, st_hi):
            for st in range(st_lo, st_hi):
                pv = psum.tile([128, D], FP, tag="sc", name="pv")
                for kj in range(2):
                    nc.tensor.matmul(
                        pv,
                        lhsT=cT[kj][:, 128 * st : 128 * st + 128],
                        rhs=wkv[kj][:, D : 2 * D],
                        start=(kj == 0),
                        stop=(kj == 1),
                    )
                dst = vP[:, VW * st : VW * st + VW].rearrange(
                    "p (h w) -> p h w", h=H
                )[:, :, 0:Dh]
                nc.vector.tensor_copy(dst, pv.rearrange("p (h w) -> p h w", h=H))

        ones_cols = vP.rearrange("p (s h w) -> p s h w", s=NST, h=H)[:, :, :, Dh : Dh + 1]
        nc.vector.memset(ones_cols, 1.0)
        qk_proj(0)
        v_proj(0, 8)
        qk_proj(1)
        v_proj(8, NST)

        # ---- Phase 3: fused attention ----
        # S-tiles processed in pairs: grouping the K=32 score matmuls and the
        # K=128 attn@v matmuls into runs halves the PE K-geometry switches
        # (~200ns each)
        dn_tiles = []
        for h in range(H):
            jj, aa = h // 2, h % 2
            tt_, bb_ = TRIP[h]
            acc = psum.tile([Dh + 1, TL], FP, tag="acc", name="acc", bufs=1)
            GRP = 4
            for sp in range(NST // GRP):
                sts = range(GRP * sp, GRP * sp + GRP)
                scs = []
                for st in sts:
                    sc = psum.tile([128, TL], FP, tag="sc", name="sc")
                    for nt in range(2):
                        nc.tensor.matmul(
                            sc[:, 512 * nt : 512 * nt + 512],
                            lhsT=kT[jj][32 * aa : 32 * aa + 32, 128 * st : 128 * st + 128],
                            rhs=qT[jj][32 * aa : 32 * aa + 32, 512 * nt : 512 * nt + 512],
                            start=True,
                            stop=True,
                            skip_group_check=True,
                        )
                    scs.append(sc)
                ats = []
                for sc in scs:
                    at = attnp.tile([128, TL], F16, tag="at", name="at")
                    nc.scalar.activation(
                        at, sc, mybir.ActivationFunctionType.Exp, scale=SCALE
                    )
                    ats.append(at)
                for i, st in enumerate(sts):
                    at = ats[i]
                    for nt in range(2):
                        nc.tensor.matmul(
                            acc[:, 512 * nt : 512 * nt + 512],
                            lhsT=vP[:, VW * st + (Dh + 1) * h : VW * st + (Dh + 1) * h + Dh + 1],
                            rhs=at[:, 512 * nt : 512 * nt + 512],
                            start=(st == 0),
                            stop=(st == NST - 1),
                            skip_group_check=True,
                        )
            nc.vector.tensor_copy(dumpT[tt_][32 * bb_ : 32 * bb_ + 32, :], acc[0:Dh, :])
            dnt = dnp.tile([1, TL], FP, tag="dn", name="dn")
            nc.vector.tensor_copy(dnt, acc[Dh : Dh + 1, :])
            dn_tiles.append(dnt)
            nc.sync.dma_start(out=dnscr[h : h + 1, :], in_=dnt)
            if bb_ == len(TRIP_HEADS[tt_]) - 1:
                t = tt_
                heads = TRIP_HEADS[t]
                nr = 32 * len(heads)
                for bi, hh in enumerate(heads):
                    nc.sync.dma_start(
                        out=rden[t][32 * bi : 32 * bi + 32, :],
                        in_=dnscr[hh : hh + 1, :].partition_broadcast(32),
                    )
                nc.vector.reciprocal_approx_fast(rcp[t][0:nr, :], rden[t][0:nr, :])
                for bi in range(len(heads)):
                    nc.vector.tensor_mul(
                        outTh[t][32 * bi : 32 * bi + 32, :],
                        dumpT[t][32 * bi : 32 * bi + 32, :],
                        rcp[t][32 * bi : 32 * bi + 32, :],
                    )

        # ---- Phase 4: output projection (normalization ran in phase 3) ----
        for tt in range(TL // 128):
            fin = psum.tile([128, D], FP, tag="sc", name="fin")
            for t in range(3):
                nc.tensor.matmul(
                    fin,
                    lhsT=outTh[t][0 : wo_rows[t], 128 * tt : 128 * tt + 128],
                    rhs=woutg[t],
                    start=(t == 0),
                    stop=(t == 2),
                )
            outs = fstage.tile([128, D], FP, tag="outs", name="outs")
            nc.vector.tensor_add(outs, fin, bias_c)
            nc.sync.dma_start(out=out_d[128 * tt : 128 * tt + 128, :], in_=outs)

    nc.compile()
    return nc


_NC = None


def kernel(**inputs):
    global _NC
    x = np.ascontiguousarray(inputs["x"], dtype=np.float32)
    context = np.ascontiguousarray(inputs["context"], dtype=np.float32)
    w_q = np.ascontiguousarray(inputs["w_q"], dtype=np.float32)
    w_kv = np.ascontiguousarray(inputs["w_kv"], dtype=np.float32)
    w_out = np.ascontiguousarray(inputs["w_out"], dtype=np.float32)
    b_out = np.ascontiguousarray(inputs["b_out"], dtype=np.float32).reshape(1, D)

    if _NC is None:
        _NC = build_bass()
    nc = _NC

    in_maps = []
    for c in range(8):
        b, half = c // 2, c % 2
        in_maps.append(
            {
                "ident": np.eye(128, dtype=np.float32),
                "x": np.ascontiguousarray(x[b, TL * half : TL * half + TL, :]),
                "context": np.ascontiguousarray(context[b]),
                "w_q": w_q,
                "w_kv": w_kv,
                "w_out": w_out,
                "b_out": b_out,
            }
        )
    res = run_bass_kernel_spmd(nc, in_maps, core_ids=list(range(8)))
    out = np.empty((B, T, D), dtype=np.float32)
    for c in range(8):
        b, half = c // 2, c % 2
        out[b, TL * half : TL * half + TL, :] = res.results[c]["out"]
    return out


if __name__ == "__main__":
    rng = np.random.default_rng(0)
    ins = {
        "x": rng.standard_normal((B, T, D), dtype=np.float32),
        "context": rng.standard_normal((B, S, D), dtype=np.float32),
        "w_q": rng.standard_normal((D, D), dtype=np.float32) * D**-0.5,
        "w_kv": rng.standard_normal((D, 2 * D), dtype=np.float32) * D**-0.5,
        "w_out": rng.standard_normal((D, D), dtype=np.float32) * D**-0.5,
        "b_out": rng.standard_normal((D,), dtype=np.float32) * 0.01,
    }
    out = kernel(**ins)
    print(out.shape, out.dtype, np.abs(out).mean())



# revision 2
# speedup vs baseline: 1.0625x; 1.0625x over previous
"""Trainium2 Bass kernel for CrossAttention (B=4, T=2048, S=4096, D=256, H=8, Dh=32).

Sharding: 8 cores = 4 batches x 2 T-halves (1024 query rows each, all heads).
No collectives; host concatenates.

Key structure vs the previous version (363us):
 - heads stacked 4-per-128-partitions (head 4g+i at partitions 32i of group g)
   so the K=32 score matmuls run CONCURRENTLY in separate PE row-groups via
   tile_position=(32i,0) (row-tiling, ~2x for a pair) and the M=33 attn@v
   matmuls run concurrently in col-groups 0/2 via tile_position=(0,{0,64}).
 - exp is split across engines: ACT does exact exp(scale*x); DVE+GpSimd lanes
   compute a Schraudolph-style approximate exp: y=fp16(A*s+B) lands on the
   integer grid [2048,4096) (fp16 rounding = float->int for free), then a
   uint16 logical-shift-left-5 turns y's bit pattern into 2^(t-15)*(1+frac).
   Positive, fp16, feeds attn@v directly. Share tuned so rel-err stays ~1.3e-2.
 - attention T is processed in two 512-col halves per (pair, st) so PSUM fits:
   sc ring [128,1024]x3 (12KB) + acc [128,1024]x1 (4KB) = 16KB.
 - denominator via the ones-column in the packed v' (row 32/96 of acc), DMA'd
   straight from PSUM to a DRAM scratch, broadcast back for normalization.
"""

import sys

if "/opt/trn_rl_repo" not in sys.path:
    sys.path.insert(0, "/opt/trn_rl_repo")

from contextlib import ExitStack

import numpy as np

import concourse.bass as bass
import concourse.tile as tile
from concourse import bacc
from concourse import mybir
from concourse.bass_utils import run_bass_kernel_spmd

B, T, S, D, H, Dh = 4, 2048, 4096, 256, 8, 32
TL = T // 2          # 1024 query rows per core
TH = TL // 2         # 512-col half for attention PSUM accumulation
NST = S // 128       # 32 S-tiles
SCALE = Dh ** -0.5
FP = mybir.dt.float32
F16 = mybir.dt.float16
U16 = mybir.dt.uint16
VW = H * (Dh + 1)    # 264 packed v' cols per S-tile

# Schraudolph exp constants (see module docstring); sigma tuned for zero-mean
# relative error so the softmax normalization cancels the bias.
LOG2E = 1.4426950408889634
SIGMA = 0.058
EXP_A = float(64.0 * LOG2E * SCALE)
EXP_B = float(3008.0 - 64.0 * SIGMA)
# lane pattern: which exp tiles go to the DVE approximate lane
DVE_LANE = (0, 3, 6)   # of 9 -> 33% approximate
LANE_MOD = 9


def build_bass():
    nc = bacc.Bacc()
    ident_d = nc.declare_dram_parameter("ident", [128, 128], FP, isOutput=False)
    x_d = nc.declare_dram_parameter("x", [TL, D], FP, isOutput=False)
    ctx_d = nc.declare_dram_parameter("context", [S, D], FP, isOutput=False)
    wq_d = nc.declare_dram_parameter("w_q", [D, D], FP, isOutput=False)
    wkv_d = nc.declare_dram_parameter("w_kv", [D, 2 * D], FP, isOutput=False)
    wout_d = nc.declare_dram_parameter("w_out", [D, D], FP, isOutput=False)
    bout_d = nc.declare_dram_parameter("b_out", [1, D], FP, isOutput=False)
    out_d = nc.declare_dram_parameter("out", [TL, D], FP, isOutput=True)
    dnscr = nc.dram_tensor("dnscratch", [H, TL], FP)

    with tile.TileContext(nc) as tc, ExitStack() as ctx:
        consts = ctx.enter_context(tc.tile_pool(name="consts", bufs=1))
        persist = ctx.enter_context(tc.tile_pool(name="persist", bufs=1))

        identity = consts.tile([128, 128], FP, tag="identity", name="identity")
        idh = consts.tile([128, 128], F16, tag="idh", name="idh")

        wqh = [persist.tile([128, D], F16, tag=f"wqh{j}", name=f"wqh{j}") for j in range(2)]
        wkvh = [persist.tile([128, 2 * D], F16, tag=f"wkvh{j}", name=f"wkvh{j}") for j in range(2)]
        woh = [persist.tile([128, D], F16, tag=f"woh{g}", name=f"woh{g}") for g in range(2)]
        bias_c = persist.tile([128, D], FP, tag="bias_c", name="bias_c")
        xT = [persist.tile([128, TL], F16, tag=f"xT{j}", name=f"xT{j}") for j in range(2)]
        cT = [persist.tile([128, S], F16, tag=f"cT{j}", name=f"cT{j}") for j in range(2)]
        qT4 = [persist.tile([128, TL], F16, tag=f"qT4{g}", name=f"qT4{g}") for g in range(2)]
        kT4 = [persist.tile([128, S], F16, tag=f"kT4{g}", name=f"kT4{g}") for g in range(2)]
        vP = persist.tile([128, NST * VW], F16, tag="vP", name="vP")
        dT4 = [persist.tile([128, TL], F16, tag=f"dT4{g}", name=f"dT4{g}") for g in range(2)]
        outT4 = [persist.tile([128, TL], F16, tag=f"oT4{g}", name=f"oT4{g}") for g in range(2)]
        rden4 = [persist.tile([128, TL], FP, tag=f"rd4{g}", name=f"rd4{g}") for g in range(2)]
        rcp4 = [persist.tile([128, TL], FP, tag=f"rc4{g}", name=f"rc4{g}") for g in range(2)]

        # ---- prologue pools (scoped; closed before attention pools open) ----
        early = tc.tile_pool(name="early", bufs=1)
        ep = early.__enter__()
        x_all = ep.tile([128, TL // 128, D], FP, tag="x_all", name="x_all")
        xh = ep.tile([128, TL // 128, D], F16, tag="xh", name="xh")
        wstage = [ep.tile([128, 4 * D], FP, tag=f"wst{j}", name=f"wst{j}") for j in range(2)]
        cstage = tc.tile_pool(name="cstage", bufs=2)
        cp = cstage.__enter__()
        ppsum = tc.tile_pool(name="ppsum", bufs=1, space="PSUM")
        pp = ppsum.__enter__()

        # ---- phase 0: loads + fp16 weight conversion ----
        nc.sync.dma_start(out=identity, in_=ident_d[:, :])
        for j in range(2):
            nc.sync.dma_start(out=wstage[j][:, 0:D], in_=wq_d[128 * j : 128 * j + 128, :])
            nc.sync.dma_start(out=wstage[j][:, D : 3 * D], in_=wkv_d[128 * j : 128 * j + 128, :])
            nc.sync.dma_start(out=wstage[j][:, 3 * D : 4 * D], in_=wout_d[128 * j : 128 * j + 128, :])
            nc.vector.tensor_copy(wqh[j], wstage[j][:, 0:D])
            nc.vector.tensor_copy(wkvh[j], wstage[j][:, D : 3 * D])
            nc.vector.tensor_copy(woh[j], wstage[j][:, 3 * D : 4 * D])
        nc.sync.dma_start(out=x_all, in_=x_d.rearrange("(t p) d -> p t d", p=128))
        nc.sync.dma_start(out=bias_c, in_=bout_d[0:1, :].partition_broadcast(128))
        nc.vector.tensor_copy(idh, identity)
        nc.gpsimd.tensor_copy(xh, x_all)

        dump_rr = [0]

        def dump(dst, src):
            # PSUM->SBUF dumps: alternate ACT (idle pre-attention) and DVE
            if dump_rr[0] % 2 == 0:
                nc.scalar.copy(dst, src)
            else:
                nc.vector.tensor_copy(dst, src)
            dump_rr[0] += 1

        # ---- phase 1: x transposes + q projection ----
        for t in range(TL // 128):
            pt = pp.tile([128, 256], F16, tag="tp", name="pt", bufs=2)
            for j in range(2):
                nc.tensor.transpose(pt[:, 128 * j : 128 * j + 128], xh[:, t, 128 * j : 128 * j + 128], idh)
            for j in range(2):
                dump(xT[j][:, 128 * t : 128 * t + 128], pt[:, 128 * j : 128 * j + 128])
        for g in range(2):
            for nt in range(TL // 512):
                pq = pp.tile([128, 512], FP, tag="pj", name="pq", bufs=2)
                for kj in range(2):
                    nc.tensor.matmul(
                        pq,
                        lhsT=wqh[kj][:, 128 * g : 128 * g + 128],
                        rhs=xT[kj][:, 512 * nt : 512 * nt + 512],
                        start=(kj == 0),
                        stop=(kj == 1),
                    )
                dump(qT4[g][:, 512 * nt : 512 * nt + 512], pq)

        # ---- phase 2: context streaming: transpose + k/v projections ----
        NCC = 8  # chunks of 512 context rows
        ctx_r = ctx_d.rearrange("(t p) d -> p t d", p=128)
        for cc in range(NCC):
            cst = cp.tile([128, 4, D], FP, tag="cst", name="cst", bufs=2)
            nc.sync.dma_start(out=cst, in_=ctx_r[:, 4 * cc : 4 * cc + 4, :])
            chh = cp.tile([128, 4, D], F16, tag="chh", name="chh", bufs=2)
            nc.gpsimd.tensor_copy(chh, cst)
            for tt in range(4):
                pt = pp.tile([128, 256], F16, tag="tp", name="pt", bufs=2)
                for j in range(2):
                    nc.tensor.transpose(pt[:, 128 * j : 128 * j + 128], chh[:, tt, 128 * j : 128 * j + 128], idh)
                for j in range(2):
                    dump(cT[j][:, 512 * cc + 128 * tt : 512 * cc + 128 * tt + 128], pt[:, 128 * j : 128 * j + 128])
            for g in range(2):
                pk = pp.tile([128, 512], FP, tag="pj", name="pk", bufs=2)
                for kj in range(2):
                    nc.tensor.matmul(
                        pk,
                        lhsT=wkvh[kj][:, 128 * g : 128 * g + 128],
                        rhs=cT[kj][:, 512 * cc : 512 * cc + 512],
                        start=(kj == 0),
                        stop=(kj == 1),
                    )
                dump(kT4[g][:, 512 * cc : 512 * cc + 512], pk)
            for st in range(4 * cc, 4 * cc + 4):
                pv = pp.tile([128, D], FP, tag="pv", name="pv", bufs=2)
                for kj in range(2):
                    nc.tensor.matmul(
                        pv,
                        lhsT=cT[kj][:, 128 * st : 128 * st + 128],
                        rhs=wkvh[kj][:, D : 2 * D],
                        start=(kj == 0),
                        stop=(kj == 1),
                    )
                dst = vP[:, VW * st : VW * st + VW].rearrange("p (h w) -> p h w", h=H)[:, :, 0:Dh]
                dump(dst, pv.rearrange("p (h w) -> p h w", h=H))

        ones_cols = vP.rearrange("p (s h w) -> p s h w", s=NST, h=H)[:, :, :, Dh : Dh + 1]
        nc.vector.memset(ones_cols, 1.0)

        ppsum.__exit__(None, None, None)
        cstage.__exit__(None, None, None)
        early.__exit__(None, None, None)

        # ---- phase 3: fused attention ----
        attnp = ctx.enter_context(tc.tile_pool(name="attn", bufs=6))
        apsum = ctx.enter_context(tc.tile_pool(name="apsum", bufs=1, space="PSUM"))
        dnp = ctx.enter_context(tc.tile_pool(name="dnp", bufs=4))

        exp_n = [0]

        def emit_exp(at_t, sc_t):
            lane_dve = (exp_n[0] % LANE_MOD) in DVE_LANE
            exp_n[0] += 1
            if lane_dve:
                nc.vector.tensor_scalar(
                    at_t, sc_t, EXP_A, EXP_B, mybir.AluOpType.mult, mybir.AluOpType.add
                )
                atu = at_t.bitcast(U16)
                nc.vector.tensor_scalar(
                    atu, atu, 5, None, mybir.AluOpType.logical_shift_left
                )
            else:
                nc.scalar.activation(at_t, sc_t, mybir.ActivationFunctionType.Exp, scale=SCALE)

        for pair in range(4):
            g, i0 = pair // 2, 2 * (pair % 2)
            hA, hB = 4 * g + i0, 4 * g + i0 + 1
            rA, rB = 32 * i0, 32 * i0 + 32
            acc = apsum.tile([128, TL], FP, tag="acc", name="acc", bufs=1)
            for st in range(NST):
                sc_t = []
                for th in range(2):
                    sc = apsum.tile([128, 2 * TH], FP, tag="sc", name="sc", bufs=3)
                    nc.tensor.matmul(
                        sc[:, 0:TH],
                        lhsT=kT4[g][rA : rA + 32, 128 * st : 128 * st + 128],
                        rhs=qT4[g][rA : rA + 32, TH * th : TH * th + TH],
                        start=True,
                        stop=True,
                        tile_position=(rA, 0),
                        skip_group_check=True,
                    )
                    nc.tensor.matmul(
                        sc[:, TH : 2 * TH],
                        lhsT=kT4[g][rB : rB + 32, 128 * st : 128 * st + 128],
                        rhs=qT4[g][rB : rB + 32, TH * th : TH * th + TH],
                        start=True,
                        stop=True,
                        tile_position=(rB, 0),
                        skip_group_check=True,
                    )
                    sc_t.append(sc)
                at_t = []
                for th in range(2):
                    at = attnp.tile([128, 2 * TH], F16, tag="at", name="at")
                    emit_exp(at, sc_t[th])
                    at_t.append(at)
                for th in range(2):
                    nc.tensor.matmul(
                        acc[0:33, TH * th : TH * th + TH],
                        lhsT=vP[:, VW * st + 33 * hA : VW * st + 33 * hA + 33],
                        rhs=at_t[th][:, 0:TH],
                        start=(st == 0),
                        stop=(st == NST - 1),
                        tile_position=(0, 0),
                        skip_group_check=True,
                    )
                    nc.tensor.matmul(
                        acc[64:97, TH * th : TH * th + TH],
                        lhsT=vP[:, VW * st + 33 * hB : VW * st + 33 * hB + 33],
                        rhs=at_t[th][:, TH : 2 * TH],
                        start=(st == 0),
                        stop=(st == NST - 1),
                        tile_position=(0, 64),
                        skip_group_check=True,
                    )
            # pair epilogue: dump 32 v-rows per head; denominator row -> DRAM
            nc.vector.tensor_copy(dT4[g][rA : rA + 32, :], acc[0:32, :])
            nc.vector.tensor_copy(dT4[g][rA + 32 : rA + 64, :], acc[64:96, :])
            dnA = dnp.tile([1, TL], FP, tag="dn", name="dn")
            dnB = dnp.tile([1, TL], FP, tag="dn", name="dn")
            nc.scalar.copy(dnA, acc[32:33, :])
            nc.scalar.copy(dnB, acc[96:97, :])
            nc.sync.dma_start(out=dnscr[hA : hA + 1, :], in_=dnA)
            nc.sync.dma_start(out=dnscr[hB : hB + 1, :], in_=dnB)

        # ---- phase 4: normalization + output projection ----
        for g in range(2):
            for i in range(4):
                nc.sync.dma_start(
                    out=rden4[g][32 * i : 32 * i + 32, :],
                    in_=dnscr[4 * g + i : 4 * g + i + 1, :].partition_broadcast(32),
                )
            nc.vector.reciprocal_approx_fast(rcp4[g], rden4[g])
            nc.vector.tensor_mul(outT4[g], dT4[g], rcp4[g])
        fstage = ctx.enter_context(tc.tile_pool(name="fstage", bufs=4))
        for tt in range(TL // 128):
            fin = apsum.tile([128, 2 * TH], FP, tag="sc", name="fin", bufs=3)
            for g in range(2):
                nc.tensor.matmul(
                    fin[:, 0:D],
                    lhsT=outT4[g][:, 128 * tt : 128 * tt + 128],
                    rhs=woh[g],
                    start=(g == 0),
                    stop=(g == 1),
                )
            outs = fstage.tile([128, D], FP, tag="outs", name="outs")
            nc.vector.tensor_add(outs, fin[:, 0:D], bias_c)
            nc.sync.dma_start(out=out_d[128 * tt : 128 * tt + 128, :], in_=outs)

    nc.compile()
    return nc


_NC = None


def kernel(**inputs):
    global _NC
    x = np.ascontiguousarray(inputs["x"], dtype=np.float32)
    context = np.ascontiguousarray(inputs["context"], dtype=np.float32)
    w_q = np.ascontiguousarray(inputs["w_q"], dtype=np.float32)
    w_kv = np.ascontiguousarray(inputs["w_kv"], dtype=np.float32)
    w_out = np.ascontiguousarray(inputs["w_out"], dtype=np.float32)
    b_out = np.ascontiguousarray(inputs["b_out"], dtype=np.float32).reshape(1, D)

    if _NC is None:
        _NC = build_bass()
    nc = _NC

    in_maps = []
    for c in range(8):
        b, half = c // 2, c % 2
        in_maps.append(
            {
                "ident": np.eye(128, dtype=np.float32),
                "x": np.ascontiguousarray(x[b, TL * half : TL * half + TL, :]),
                "context": np.ascontiguousarray(context[b]),
                "w_q": w_q,
                "w_kv": w_kv,
                "w_out": w_out,
                "b_out": b_out,
            }
        )
    res = run_bass_kernel_spmd(nc, in_maps, core_ids=list(range(8)))
    out = np.empty((B, T, D), dtype=np.float32)
    for c in range(8):
        b, half = c // 2, c % 2
        out[b, TL * half : TL * half + TL, :] = res.results[c]["out"]
    return out


if __name__ == "__main__":
    rng = np.random.default_rng(0)
    ins = {
        "x": rng.standard_normal((B, T, D), dtype=np.float32),
        "context": rng.standard_normal((B, S, D), dtype=np.float32),
        "w_q": rng.standard_normal((D, D), dtype=np.float32) * D**-0.5,
        "w_kv": rng.standard_normal((D, 2 * D), dtype=np.float32) * D**-0.5,
        "w_out": rng.standard_normal((D, D), dtype=np.float32) * D**-0.5,
        "b_out": rng.standard_normal((D,), dtype=np.float32) * 0.01,
    }
    out = kernel(**ins)

    # numpy reference
    x, c = ins["x"], ins["context"]
    q = (x @ ins["w_q"]).reshape(B, T, H, Dh)
    kv = c @ ins["w_kv"]
    k, v = kv[..., :256].reshape(B, S, H, Dh), kv[..., 256:].reshape(B, S, H, Dh)
    att = np.einsum("bthd,bshd->bhts", q, k) * SCALE
    att = np.exp(att - att.max(-1, keepdims=True))
    att /= att.sum(-1, keepdims=True)
    ref = np.einsum("bhts,bshd->bthd", att, v).reshape(B, T, 256) @ ins["w_out"] + ins["b_out"]
    rel = np.linalg.norm(out - ref) / np.linalg.norm(ref)
    print(f"shape {out.shape} rel_err {rel:.3e}")


# revision 3
# speedup vs baseline: 1.0663x; 1.0035x over previous
"""Trainium2 Bass kernel for CrossAttention (B=4, T=2048, S=4096, D=256, H=8, Dh=32).

Sharding: 8 cores = 4 batches x 2 T-halves (1024 query rows each, all heads).
No collectives; host concatenates.

Key structure vs the previous version (363us):
 - heads stacked 4-per-128-partitions (head 4g+i at partitions 32i of group g)
   so the K=32 score matmuls run CONCURRENTLY in separate PE row-groups via
   tile_position=(32i,0) (row-tiling, ~2x for a pair) and the M=33 attn@v
   matmuls run concurrently in col-groups 0/2 via tile_position=(0,{0,64}).
 - exp is split across engines: ACT does exact exp(scale*x); DVE+GpSimd lanes
   compute a Schraudolph-style approximate exp: y=fp16(A*s+B) lands on the
   integer grid [2048,4096) (fp16 rounding = float->int for free), then a
   uint16 logical-shift-left-5 turns y's bit pattern into 2^(t-15)*(1+frac).
   Positive, fp16, feeds attn@v directly. Share tuned so rel-err stays ~1.3e-2.
 - attention T is processed in two 512-col halves per (pair, st) so PSUM fits:
   sc ring [128,1024]x3 (12KB) + acc [128,1024]x1 (4KB) = 16KB.
 - denominator via the ones-column in the packed v' (row 32/96 of acc), DMA'd
   straight from PSUM to a DRAM scratch, broadcast back for normalization.
"""

import sys

if "/opt/trn_rl_repo" not in sys.path:
    sys.path.insert(0, "/opt/trn_rl_repo")

from contextlib import ExitStack

import numpy as np

import concourse.bass as bass
import concourse.tile as tile
from concourse import bacc
from concourse import mybir
from concourse.bass_utils import run_bass_kernel_spmd

B, T, S, D, H, Dh = 4, 2048, 4096, 256, 8, 32
TL = T // 2          # 1024 query rows per core
TH = TL // 2         # 512-col half for attention PSUM accumulation
NST = S // 128       # 32 S-tiles
SCALE = Dh ** -0.5
FP = mybir.dt.float32
F16 = mybir.dt.float16
U16 = mybir.dt.uint16
VW = H * (Dh + 1)    # 264 packed v' cols per S-tile

# Schraudolph exp constants (see module docstring); sigma tuned for zero-mean
# relative error so the softmax normalization cancels the bias.
LOG2E = 1.4426950408889634
SIGMA = 0.058
EXP_A = float(64.0 * LOG2E * SCALE)
EXP_B = float(3008.0 - 64.0 * SIGMA)
# lane pattern: which exp tiles go to the DVE approximate lane
DVE_LANE = (0, 2, 5, 7, 10, 12, 14, 16)   # 8 of 19 -> 42% approximate
LANE_MOD = 19


def build_bass():
    nc = bacc.Bacc()
    ident_d = nc.declare_dram_parameter("ident", [128, 128], FP, isOutput=False)
    x_d = nc.declare_dram_parameter("x", [TL, D], FP, isOutput=False)
    ctx_d = nc.declare_dram_parameter("context", [S, D], FP, isOutput=False)
    wq_d = nc.declare_dram_parameter("w_q", [D, D], FP, isOutput=False)
    wkv_d = nc.declare_dram_parameter("w_kv", [D, 2 * D], FP, isOutput=False)
    wout_d = nc.declare_dram_parameter("w_out", [D, D], FP, isOutput=False)
    bout_d = nc.declare_dram_parameter("b_out", [1, D], FP, isOutput=False)
    out_d = nc.declare_dram_parameter("out", [TL, D], FP, isOutput=True)
    dnscr = nc.dram_tensor("dnscratch", [H, TL], FP)

    with tile.TileContext(nc) as tc, ExitStack() as ctx:
        consts = ctx.enter_context(tc.tile_pool(name="consts", bufs=1))
        persist = ctx.enter_context(tc.tile_pool(name="persist", bufs=1))

        identity = consts.tile([128, 128], FP, tag="identity", name="identity")
        idh = consts.tile([128, 128], F16, tag="idh", name="idh")

        wqh = [persist.tile([128, D], F16, tag=f"wqh{j}", name=f"wqh{j}") for j in range(2)]
        wkvh = [persist.tile([128, 2 * D], F16, tag=f"wkvh{j}", name=f"wkvh{j}") for j in range(2)]
        woh = [persist.tile([128, D], F16, tag=f"woh{g}", name=f"woh{g}") for g in range(2)]
        bias_c = persist.tile([128, D], FP, tag="bias_c", name="bias_c")
        xT = [persist.tile([128, TL], F16, tag=f"xT{j}", name=f"xT{j}") for j in range(2)]
        cT = [persist.tile([128, S], F16, tag=f"cT{j}", name=f"cT{j}") for j in range(2)]
        qT4 = [persist.tile([128, TL], F16, tag=f"qT4{g}", name=f"qT4{g}") for g in range(2)]
        kT4 = [persist.tile([128, S], F16, tag=f"kT4{g}", name=f"kT4{g}") for g in range(2)]
        vP = persist.tile([128, NST * VW], F16, tag="vP", name="vP")
        dT4 = [persist.tile([128, TL], F16, tag=f"dT4{g}", name=f"dT4{g}") for g in range(2)]
        outT4 = [persist.tile([128, TL], F16, tag=f"oT4{g}", name=f"oT4{g}") for g in range(2)]
        rden4 = [persist.tile([128, TL], FP, tag=f"rd4{g}", name=f"rd4{g}") for g in range(2)]
        rcp4 = [persist.tile([128, TL], FP, tag=f"rc4{g}", name=f"rc4{g}") for g in range(2)]

        # ---- prologue pools (scoped; closed before attention pools open) ----
        early = tc.tile_pool(name="early", bufs=1)
        ep = early.__enter__()
        x_all = ep.tile([128, TL // 128, D], FP, tag="x_all", name="x_all")
        xh = ep.tile([128, TL // 128, D], F16, tag="xh", name="xh")
        wstage = [ep.tile([128, 4 * D], FP, tag=f"wst{j}", name=f"wst{j}") for j in range(2)]
        cstage = tc.tile_pool(name="cstage", bufs=2)
        cp = cstage.__enter__()
        ppsum = tc.tile_pool(name="ppsum", bufs=1, space="PSUM")
        pp = ppsum.__enter__()

        # ---- phase 0: loads + fp16 weight conversion ----
        nc.sync.dma_start(out=identity, in_=ident_d[:, :])
        for j in range(2):
            nc.sync.dma_start(out=wstage[j][:, 0:D], in_=wq_d[128 * j : 128 * j + 128, :])
            nc.sync.dma_start(out=wstage[j][:, D : 3 * D], in_=wkv_d[128 * j : 128 * j + 128, :])
            nc.sync.dma_start(out=wstage[j][:, 3 * D : 4 * D], in_=wout_d[128 * j : 128 * j + 128, :])
            nc.vector.tensor_copy(wqh[j], wstage[j][:, 0:D])
            nc.vector.tensor_copy(wkvh[j], wstage[j][:, D : 3 * D])
            nc.vector.tensor_copy(woh[j], wstage[j][:, 3 * D : 4 * D])
        nc.sync.dma_start(out=x_all, in_=x_d.rearrange("(t p) d -> p t d", p=128))
        nc.sync.dma_start(out=bias_c, in_=bout_d[0:1, :].partition_broadcast(128))
        nc.vector.tensor_copy(idh, identity)
        for xc in range(4):
            nc.vector.tensor_copy(xh[:, 2 * xc : 2 * xc + 2, :], x_all[:, 2 * xc : 2 * xc + 2, :])

        dump_rr = [0]

        def dump(dst, src):
            # PSUM->SBUF dumps: alternate ACT (idle pre-attention) and DVE
            if dump_rr[0] % 2 == 0:
                nc.scalar.copy(dst, src)
            else:
                nc.vector.tensor_copy(dst, src)
            dump_rr[0] += 1

        # ---- phase 1: x transposes + q projection ----
        for t in range(TL // 128):
            pt = pp.tile([128, 256], F16, tag="tp", name="pt", bufs=2)
            for j in range(2):
                nc.tensor.transpose(pt[:, 128 * j : 128 * j + 128], xh[:, t, 128 * j : 128 * j + 128], idh)
            for j in range(2):
                dump(xT[j][:, 128 * t : 128 * t + 128], pt[:, 128 * j : 128 * j + 128])
        for g in range(2):
            for nt in range(TL // 512):
                pq = pp.tile([128, 512], FP, tag="pj", name="pq", bufs=2)
                for kj in range(2):
                    nc.tensor.matmul(
                        pq,
                        lhsT=wqh[kj][:, 128 * g : 128 * g + 128],
                        rhs=xT[kj][:, 512 * nt : 512 * nt + 512],
                        start=(kj == 0),
                        stop=(kj == 1),
                    )
                dump(qT4[g][:, 512 * nt : 512 * nt + 512], pq)

        # ---- phase 2: context streaming: transpose + k/v projections ----
        NCC = 8  # chunks of 512 context rows
        ctx_r = ctx_d.rearrange("(t p) d -> p t d", p=128)
        for cc in range(NCC):
            cst = cp.tile([128, 4, D], FP, tag="cst", name="cst", bufs=2)
            nc.sync.dma_start(out=cst, in_=ctx_r[:, 4 * cc : 4 * cc + 4, :])
            chh = cp.tile([128, 4, D], F16, tag="chh", name="chh", bufs=2)
            for q4 in range(2):
                if (2 * cc + q4) % 2 == 0:
                    nc.vector.tensor_copy(chh[:, 2 * q4 : 2 * q4 + 2, :], cst[:, 2 * q4 : 2 * q4 + 2, :])
                else:
                    nc.scalar.copy(chh[:, 2 * q4 : 2 * q4 + 2, :], cst[:, 2 * q4 : 2 * q4 + 2, :])
            for tt in range(4):
                pt = pp.tile([128, 256], F16, tag="tp", name="pt", bufs=2)
                for j in range(2):
                    nc.tensor.transpose(pt[:, 128 * j : 128 * j + 128], chh[:, tt, 128 * j : 128 * j + 128], idh)
                for j in range(2):
                    dump(cT[j][:, 512 * cc + 128 * tt : 512 * cc + 128 * tt + 128], pt[:, 128 * j : 128 * j + 128])
            for g in range(2):
                pk = pp.tile([128, 512], FP, tag="pj", name="pk", bufs=2)
                for kj in range(2):
                    nc.tensor.matmul(
                        pk,
                        lhsT=wkvh[kj][:, 128 * g : 128 * g + 128],
                        rhs=cT[kj][:, 512 * cc : 512 * cc + 512],
                        start=(kj == 0),
                        stop=(kj == 1),
                    )
                dump(kT4[g][:, 512 * cc : 512 * cc + 512], pk)
            for st in range(4 * cc, 4 * cc + 4):
                pv = pp.tile([128, D], FP, tag="pv", name="pv", bufs=2)
                for kj in range(2):
                    nc.tensor.matmul(
                        pv,
                        lhsT=cT[kj][:, 128 * st : 128 * st + 128],
                        rhs=wkvh[kj][:, D : 2 * D],
                        start=(kj == 0),
                        stop=(kj == 1),
                    )
                dst = vP[:, VW * st : VW * st + VW].rearrange("p (h w) -> p h w", h=H)[:, :, 0:Dh]
                dump(dst, pv.rearrange("p (h w) -> p h w", h=H))

        ones_cols = vP.rearrange("p (s h w) -> p s h w", s=NST, h=H)[:, :, :, Dh : Dh + 1]
        nc.vector.memset(ones_cols, 1.0)

        ppsum.__exit__(None, None, None)
        cstage.__exit__(None, None, None)
        early.__exit__(None, None, None)

        # ---- phase 3: fused attention ----
        attnp = ctx.enter_context(tc.tile_pool(name="attn", bufs=6))
        apsum = ctx.enter_context(tc.tile_pool(name="apsum", bufs=1, space="PSUM"))
        dnp = ctx.enter_context(tc.tile_pool(name="dnp", bufs=4))

        exp_n = [0]

        def emit_exp(at_t, sc_t):
            lane_dve = (exp_n[0] % LANE_MOD) in DVE_LANE
            exp_n[0] += 1
            if lane_dve:
                nc.vector.tensor_scalar(
                    at_t, sc_t, EXP_A, EXP_B, mybir.AluOpType.mult, mybir.AluOpType.add
                )
                atu = at_t.bitcast(U16)
                nc.vector.tensor_scalar(
                    atu, atu, 5, None, mybir.AluOpType.logical_shift_left
                )
            else:
                nc.scalar.activation(at_t, sc_t, mybir.ActivationFunctionType.Exp, scale=SCALE)

        for pair in range(4):
            g, i0 = pair // 2, 2 * (pair % 2)
            hA, hB = 4 * g + i0, 4 * g + i0 + 1
            rA, rB = 32 * i0, 32 * i0 + 32
            acc = apsum.tile([128, TL], FP, tag="acc", name="acc", bufs=1)
            for st in range(NST):
                sc_t = []
                for th in range(2):
                    sc = apsum.tile([128, 2 * TH], FP, tag="sc", name="sc", bufs=3)
                    nc.tensor.matmul(
                        sc[:, 0:TH],
                        lhsT=kT4[g][rA : rA + 32, 128 * st : 128 * st + 128],
                        rhs=qT4[g][rA : rA + 32, TH * th : TH * th + TH],
                        start=True,
                        stop=True,
                        tile_position=(rA, 0),
                        skip_group_check=True,
                    )
                    nc.tensor.matmul(
                        sc[:, TH : 2 * TH],
                        lhsT=kT4[g][rB : rB + 32, 128 * st : 128 * st + 128],
                        rhs=qT4[g][rB : rB + 32, TH * th : TH * th + TH],
                        start=True,
                        stop=True,
                        tile_position=(rB, 0),
                        skip_group_check=True,
                    )
                    sc_t.append(sc)
                at_t = []
                for th in range(2):
                    at = attnp.tile([128, 2 * TH], F16, tag="at", name="at")
                    emit_exp(at, sc_t[th])
                    at_t.append(at)
                for th in range(2):
                    nc.tensor.matmul(
                        acc[0:33, TH * th : TH * th + TH],
                        lhsT=vP[:, VW * st + 33 * hA : VW * st + 33 * hA + 33],
                        rhs=at_t[th][:, 0:TH],
                        start=(st == 0),
                        stop=(st == NST - 1),
                        tile_position=(0, 0),
                        skip_group_check=True,
                    )
                    nc.tensor.matmul(
                        acc[64:97, TH * th : TH * th + TH],
                        lhsT=vP[:, VW * st + 33 * hB : VW * st + 33 * hB + 33],
                        rhs=at_t[th][:, TH : 2 * TH],
                        start=(st == 0),
                        stop=(st == NST - 1),
                        tile_position=(0, 64),
                        skip_group_check=True,
                    )
            # pair epilogue: dump 32 v-rows per head; denominator row -> DRAM
            nc.vector.tensor_copy(dT4[g][rA : rA + 32, :], acc[0:32, :])
            nc.vector.tensor_copy(dT4[g][rA + 32 : rA + 64, :], acc[64:96, :])
            dnA = dnp.tile([1, TL], FP, tag="dn", name="dn")
            dnB = dnp.tile([1, TL], FP, tag="dn", name="dn")
            nc.scalar.copy(dnA, acc[32:33, :])
            nc.scalar.copy(dnB, acc[96:97, :])
            nc.sync.dma_start(out=dnscr[hA : hA + 1, :], in_=dnA)
            nc.sync.dma_start(out=dnscr[hB : hB + 1, :], in_=dnB)

        # ---- phase 4: normalization + output projection ----
        for g in range(2):
            for i in range(4):
                nc.sync.dma_start(
                    out=rden4[g][32 * i : 32 * i + 32, :],
                    in_=dnscr[4 * g + i : 4 * g + i + 1, :].partition_broadcast(32),
                )
            nc.vector.reciprocal_approx_fast(rcp4[g], rden4[g])
            nc.vector.tensor_mul(outT4[g], dT4[g], rcp4[g])
        fstage = ctx.enter_context(tc.tile_pool(name="fstage", bufs=4))
        for tt in range(TL // 128):
            fin = apsum.tile([128, 2 * TH], FP, tag="sc", name="fin", bufs=3)
            for g in range(2):
                nc.tensor.matmul(
                    fin[:, 0:D],
                    lhsT=outT4[g][:, 128 * tt : 128 * tt + 128],
                    rhs=woh[g],
                    start=(g == 0),
                    stop=(g == 1),
                )
            outs = fstage.tile([128, D], FP, tag="outs", name="outs")
            nc.vector.tensor_add(outs, fin[:, 0:D], bias_c)
            nc.sync.dma_start(out=out_d[128 * tt : 128 * tt + 128, :], in_=outs)

    nc.compile()
    return nc


_NC = None


def kernel(**inputs):
    global _NC
    x = np.ascontiguousarray(inputs["x"], dtype=np.float32)
    context = np.ascontiguousarray(inputs["context"], dtype=np.float32)
    w_q = np.ascontiguousarray(inputs["w_q"], dtype=np.float32)
    w_kv = np.ascontiguousarray(inputs["w_kv"], dtype=np.float32)
    w_out = np.ascontiguousarray(inputs["w_out"], dtype=np.float32)
    b_out = np.ascontiguousarray(inputs["b_out"], dtype=np.float32).reshape(1, D)

    if _NC is None:
        _NC = build_bass()
    nc = _NC

    in_maps = []
    for c in range(8):
        b, half = c // 2, c % 2
        in_maps.append(
            {
                "ident": np.eye(128, dtype=np.float32),
                "x": np.ascontiguousarray(x[b, TL * half : TL * half + TL, :]),
                "context": np.ascontiguousarray(context[b]),
                "w_q": w_q,
                "w_kv": w_kv,
                "w_out": w_out,
                "b_out": b_out,
            }
        )
    res = run_bass_kernel_spmd(nc, in_maps, core_ids=list(range(8)))
    out = np.empty((B, T, D), dtype=np.float32)
    for c in range(8):
        b, half = c // 2, c % 2
        out[b, TL * half : TL * half + TL, :] = res.results[c]["out"]
    return out


if __name__ == "__main__":
    rng = np.random.default_rng(0)
    ins = {
        "x": rng.standard_normal((B, T, D), dtype=np.float32),
        "context": rng.standard_normal((B, S, D), dtype=np.float32),
        "w_q": rng.standard_normal((D, D), dtype=np.float32) * D**-0.5,
        "w_kv": rng.standard_normal((D, 2 * D), dtype=np.float32) * D**-0.5,
        "w_out": rng.standard_normal((D, D), dtype=np.float32) * D**-0.5,
        "b_out": rng.standard_normal((D,), dtype=np.float32) * 0.01,
    }
    out = kernel(**ins)

    # numpy reference
    x, c = ins["x"], ins["context"]
    q = (x @ ins["w_q"]).reshape(B, T, H, Dh)
    kv = c @ ins["w_kv"]
    k, v = kv[..., :256].reshape(B, S, H, Dh), kv[..., 256:].reshape(B, S, H, Dh)
    att = np.einsum("bthd,bshd->bhts", q, k) * SCALE
    att = np.exp(att - att.max(-1, keepdims=True))
    att /= att.sum(-1, keepdims=True)
    ref = np.einsum("bhts,bshd->bthd", att, v).reshape(B, T, 256) @ ins["w_out"] + ins["b_out"]
    rel = np.linalg.norm(out - ref) / np.linalg.norm(ref)
    print(f"shape {out.shape} rel_err {rel:.3e}")
